# revision 1
# baseline (speedup 1.0000x reference)
"""Causal self-attention Trainium2 kernel (8 NeuronCores).

Problem: x[4,2048,1024] @ W_attn[1024,3072] -> qkv; 16-head causal attention;
ctx @ W_proj[1024,1024]. f32 I/O.

Sharding (8 cores): data parallel over batch (4) x tensor parallel over heads
(2 groups of 8). Each core computes, for its (b, g):
  - qkv^T for its 8 heads from the full x_b (on-chip transpose of x)
  - causal attention in transposed-score layout: S^T[k,q] blocks, exp on ACT
    (no max subtraction: scores ~ N(0,1), exp never overflows fp32),
    causal mask via affine_select, softmax denominator via a ones-column
    appended to V (ctx^T row 64 accumulates sum_k P), normalize ctx^T columns
  - partial out = ctx_g @ W_proj[g*512:(g+1)*512, :]  (f32 [2048,1024])
Host gather: out_b = partial(b,0) + partial(b,1) + b_proj.

Matmul compute dtype bf16 (fp32 PSUM accumulate).
"""

import sys

if "/opt/trn_rl_repo" not in sys.path:
    sys.path.insert(0, "/opt/trn_rl_repo")

from contextlib import ExitStack

import numpy as np

import concourse.mybir as mybir
import concourse.tile as tile
from concourse import bacc
from concourse.masks import make_identity

# geometry (hardcoded for this problem)
B, S, D = 4, 2048, 1024
H, HD = 16, 64
NCORES = 8
NH = 8            # heads per core
GC = D // 2       # per-core qkv col split = 512
ST = S // 128     # seq tiles of 128
DC = D // 128     # contraction chunks of 128
NQ = S // 512     # 512-wide q chunks
SCALE = 1.0 / np.sqrt(HD)

MDT = mybir.dt.bfloat16  # matmul compute dtype
F32 = mybir.dt.float32


def build_bass(phase="D"):
    nc = bacc.Bacc()
    dt = mybir.dt

    x = nc.declare_dram_parameter("x", [S, D], F32, isOutput=False)
    wq = nc.declare_dram_parameter("wq", [D, GC], F32, isOutput=False)
    wk = nc.declare_dram_parameter("wk", [D, GC], F32, isOutput=False)
    wv = nc.declare_dram_parameter("wv", [D, GC], F32, isOutput=False)
    bqkv = nc.declare_dram_parameter("bqkv", [3 * GC], F32, isOutput=False)  # [bq|bk|bv]
    wp = nc.declare_dram_parameter("wp", [GC, D], F32, isOutput=False)
    o = nc.declare_dram_parameter("o", [S, D], F32, isOutput=True)
    wmats = {"q": wq, "k": wk, "v": wv}

    with ExitStack() as top:
        tc = top.enter_context(tile.TileContext(nc))

        const = top.enter_context(tc.tile_pool(name="const", bufs=1))
        ident = const.tile([128, 128], MDT)
        make_identity(nc, ident[:])

        # bias tiles: one [128,1] per (mat, col-block)
        bias_t = {}
        for mi, mat in enumerate(("q", "k", "v")):
            for cb in range(GC // 128):
                t = const.tile([128, 1], F32, tag=f"bias_{mat}{cb}")
                nc.sync.dma_start(t[:], bqkv[mi * GC + cb * 128 : mi * GC + (cb + 1) * 128].rearrange("(p one) -> p one", one=1))
                bias_t[(mat, cb)] = t

        # persistent SBUF data
        data = top.enter_context(tc.tile_pool(name="data", bufs=1))
        qT = [data.tile([64, S], MDT, tag=f"qT{h}", name=f"qT{h}") for h in range(NH)]
        kT = [data.tile([64, S], MDT, tag=f"kT{h}", name=f"kT{h}") for h in range(NH)]
        vst = [data.tile([128, NH * (HD + 1)], MDT, tag=f"vst{s}", name=f"vst{s}") for s in range(ST)]
        ctxT = [data.tile([128, S], MDT, tag=f"ctxT{t}", name=f"ctxT{t}") for t in range(GC // 128)]

        # ---------------- phase A: x -> xT (bf16) ----------------
        xtv_cm = tc.tile_pool(name="xtv", bufs=1)
        xtv = xtv_cm.__enter__()
        xT = [xtv.tile([128, S], MDT, tag=f"xT{c}", name=f"xT{c}") for c in range(DC)]
        vT = [xtv.tile([128, S], MDT, tag=f"vT{cb}", name=f"vT{cb}") for cb in range(GC // 128)]
        with tc.tile_pool(name="tps", bufs=4, space="PSUM") as tps:
            with tc.tile_pool(name="xin", bufs=3) as xin:
                for s in range(ST):
                    x_sb = xin.tile([128, D], F32, tag="x_sb")
                    nc.sync.dma_start(x_sb[:], x[s * 128 : (s + 1) * 128, :])
                    x_bf = xin.tile([128, D], MDT, tag="x_bf")
                    nc.vector.tensor_copy(x_bf[:], x_sb[:])
                    for c in range(DC):
                        psA = tps.tile([128, 128], MDT, tag="psA")
                        nc.tensor.transpose(psA[:], x_bf[:, c * 128 : (c + 1) * 128], ident[:])
                        if c % 2 == 0:
                            nc.scalar.copy(xT[c][:, s * 128 : (s + 1) * 128], psA[:])
                        else:
                            nc.vector.tensor_copy(xT[c][:, s * 128 : (s + 1) * 128], psA[:])

            # ---------------- phase B: QKV^T ----------------
            with tc.tile_pool(name="wpool", bufs=2 * DC + 2) as wpool, \
                 tc.tile_pool(name="qkvps", bufs=3, space="PSUM") as qkvps_pool:
                for mat in ("q", "k", "v"):
                    for cb in range(GC // 128):
                        wbf = []
                        for d in range(DC):
                            wst = wpool.tile([128, 128], F32, tag="wst")
                            nc.sync.dma_start(wst[:], wmats[mat][d * 128 : (d + 1) * 128, cb * 128 : (cb + 1) * 128])
                            wb = wpool.tile([128, 128], MDT, tag="wbf")
                            nc.vector.tensor_copy(wb[:], wst[:])
                            wbf.append(wb)
                        for n in range(S // 512):
                            ps = qkvps_pool.tile([128, 512], F32, tag="qkvps")
                            for d in range(DC):
                                nc.tensor.matmul(
                                    ps[:], wbf[d][:], xT[d][:, n * 512 : (n + 1) * 512],
                                    start=(d == 0), stop=(d == DC - 1),
                                )
                            span = (slice(None), slice(n * 512, (n + 1) * 512))
                            bt = bias_t[(mat, cb)]
                            if mat == "v":
                                nc.scalar.activation(
                                    vT[cb][span], ps[:],
                                    mybir.ActivationFunctionType.Identity,
                                    bias=bt[:], scale=1.0,
                                )
                            else:
                                dstl = qT if mat == "q" else kT
                                for half in range(2):
                                    h = 2 * cb + half
                                    nc.scalar.activation(
                                        dstl[h][span],
                                        ps[64 * half : 64 * half + 64, :],
                                        mybir.ActivationFunctionType.Identity,
                                        bias=bt[64 * half : 64 * half + 64, :], scale=1.0,
                                    )

            # ---------------- phase B2: vT -> V natural with ones cols ----------------
            for s in range(ST):
                nc.gpsimd.memset(vst[s].rearrange("p (h u) -> p h u", u=HD + 1)[:, :, HD : HD + 1], 1.0)
            for cb in range(GC // 128):
                for s in range(ST):
                    psB = tps.tile([128, 128], MDT, tag="psA")
                    nc.tensor.transpose(psB[:], vT[cb][:, s * 128 : (s + 1) * 128], ident[:])
                    for half in range(2):
                        h = 2 * cb + half
                        eng = nc.vector.tensor_copy if half == 0 else nc.scalar.copy
                        eng(
                            vst[s][:, h * (HD + 1) : h * (HD + 1) + HD],
                            psB[:, 64 * half : 64 * half + 64],
                        )
        xtv_cm.__exit__(None, None, None)

        if phase == "B":
            with tc.tile_pool(name="dbg", bufs=2) as dbg:
                for s in range(ST):
                    d = dbg.tile([128, NH * (HD + 1)], F32, tag="dbgt")
                    nc.scalar.copy(d[:], vst[s][:])
                    nc.sync.dma_start(o[s * 128 : (s + 1) * 128, 0 : NH * (HD + 1)], d[:])

        # ---------------- phase C: attention per head ----------------
        with tc.tile_pool(name="spool", bufs=3, space="PSUM") as spool, \
             tc.tile_pool(name="cpool", bufs=5, space="PSUM") as cpool, \
             tc.tile_pool(name="ppool", bufs=6) as ppool, \
             tc.tile_pool(name="npool", bufs=4) as npool:
            for h in range(NH if phase in ("C", "D") else 0):
                ctxps = [cpool.tile([65, 512], F32, tag="ctxps", name="ctxps") for _ in range(NQ)]
                for j in range(ST):
                    c0 = j // 4
                    for c in range(c0, NQ):
                        qoff = 128 * j if c == c0 else 512 * c
                        qlen = 512 * (c + 1) - qoff
                        sps = spool.tile([128, 512], F32, tag="sps")
                        nc.tensor.matmul(
                            sps[:, :qlen],
                            kT[h][:, j * 128 : (j + 1) * 128],
                            qT[h][:, qoff : qoff + qlen],
                            start=True, stop=True,
                        )
                        pT = ppool.tile([128, 512], MDT, tag="pT")
                        nc.scalar.activation(
                            pT[:, :qlen], sps[:, :qlen],
                            mybir.ActivationFunctionType.Exp,
                            bias=0.0, scale=float(SCALE),
                        )
                        if c == c0:
                            # diagonal block: keep q >= k  (q = qoff+f = 128j+f, k = 128j+p)
                            nc.gpsimd.affine_select(
                                out=pT[:, 0:128], in_=pT[:, 0:128],
                                compare_op=mybir.AluOpType.is_ge,
                                fill=0.0, base=0,
                                pattern=[[1, 128]], channel_multiplier=-1,
                            )
                        nc.tensor.matmul(
                            ctxps[c][:, qoff - 512 * c : qoff - 512 * c + qlen],
                            vst[j][:, h * (HD + 1) : (h + 1) * (HD + 1)],
                            pT[:, :qlen],
                            start=(j == 0), stop=(j == 4 * c + 3),
                        )
                        if j == 4 * c + 3:
                            # chunk complete -> normalize columns by 1/denominator
                            rec1 = npool.tile([1, 512], F32, tag="rec1")
                            nc.vector.reciprocal(rec1[:], ctxps[c][64:65, :])
                            recb = npool.tile([64, 512], F32, tag="recb")
                            nc.gpsimd.partition_broadcast(recb[:], rec1[:])
                            nc.vector.tensor_mul(
                                ctxT[h // 2][64 * (h % 2) : 64 * (h % 2) + 64, c * 512 : (c + 1) * 512],
                                ctxps[c][0:64, :],
                                recb[:],
                            )

        if phase == "C":
            with tc.tile_pool(name="dbg2", bufs=2) as dbg2:
                for t in range(GC // 128):
                    for s in range(ST):
                        d = dbg2.tile([128, 128], F32, tag="dbg2t")
                        nc.scalar.copy(d[:], ctxT[t][:, s * 128 : (s + 1) * 128])
                        nc.sync.dma_start(o[s * 128 : (s + 1) * 128, t * 128 : (t + 1) * 128], d[:])

        # ---------------- phase D: out = ctx @ Wp ----------------
        with tc.tile_pool(name="wpstp", bufs=2) as wpstp, \
             tc.tile_pool(name="wpbp", bufs=1) as wpbp, \
             tc.tile_pool(name="opool", bufs=2) as opool, \
             tc.tile_pool(name="pps", bufs=3, space="PSUM") as pps_pool:
            if phase != "D":
                wz = wpstp.tile([128, 8], F32, tag="wz")
                nc.sync.dma_start(wz[:], wp[0:128, 0:8])
                nc.sync.dma_start(o[0:128, 0:8], wz[:])
            wpb = []
            for t in range(GC // 128 if phase == "D" else 0):
                wst = wpstp.tile([128, D], F32, tag="wpst")
                nc.sync.dma_start(wst[:], wp[t * 128 : (t + 1) * 128, :])
                wb = wpbp.tile([128, D], MDT, tag=f"wpb{t}", name=f"wpb{t}")
                nc.vector.tensor_copy(wb[:], wst[:])
                wpb.append(wb)
            for m in range(ST if phase == "D" else 0):
                out_sb = opool.tile([128, D], F32, tag="out_sb")
                for nb in range(2):
                    ps = pps_pool.tile([128, 512], F32, tag="pps")
                    for t in range(GC // 128):
                        nc.tensor.matmul(
                            ps[:],
                            ctxT[t][:, m * 128 : (m + 1) * 128],
                            wpb[t][:, nb * 512 : (nb + 1) * 512],
                            start=(t == 0), stop=(t == GC // 128 - 1),
                        )
                    nc.scalar.copy(out_sb[:, nb * 512 : (nb + 1) * 512], ps[:])
                nc.sync.dma_start(o[m * 128 : (m + 1) * 128, :], out_sb[:])

    nc.finalize()
    return nc


_NC_CACHE = None


def get_nc():
    global _NC_CACHE
    if _NC_CACHE is None:
        _NC_CACHE = build_bass()
    return _NC_CACHE


def make_in_maps(x, W_attn, b_attn, W_proj):
    x = np.asarray(x, dtype=np.float32)
    W_attn = np.ascontiguousarray(np.asarray(W_attn, dtype=np.float32))
    b_attn = np.asarray(b_attn, dtype=np.float32)
    W_proj = np.ascontiguousarray(np.asarray(W_proj, dtype=np.float32))
    in_maps = []
    for core in range(NCORES):
        b, g = divmod(core, 2)
        cs = slice(g * GC, (g + 1) * GC)
        in_maps.append({
            "x": np.ascontiguousarray(x[b]),
            "wq": np.ascontiguousarray(W_attn[:, 0 * D :][:, cs]),
            "wk": np.ascontiguousarray(W_attn[:, 1 * D :][:, cs]),
            "wv": np.ascontiguousarray(W_attn[:, 2 * D :][:, cs]),
            "bqkv": np.ascontiguousarray(np.concatenate(
                [b_attn[0 * D :][cs], b_attn[1 * D :][cs], b_attn[2 * D :][cs]]
            )),
            "wp": np.ascontiguousarray(W_proj[g * GC : (g + 1) * GC, :]),
        })
    return in_maps


def gather_output(results, b_proj):
    out = np.empty((B, S, D), dtype=np.float32)
    for b in range(B):
        out[b] = results[2 * b]["o"] + results[2 * b + 1]["o"]
    out += np.asarray(b_proj, dtype=np.float32)[None, None, :]
    return out


def kernel(x, W_attn, b_attn, W_proj, b_proj):
    from concourse.bass_utils import run_bass_kernel_spmd

    nc = get_nc()
    in_maps = make_in_maps(x, W_attn, b_attn, W_proj)
    res = run_bass_kernel_spmd(nc, in_maps, list(range(NCORES)))
    return gather_output(res.results, b_proj)



# revision 2
# speedup vs baseline: 1.9216x; 1.9216x over previous
"""Causal self-attention Trainium2 kernel (8 NeuronCores).

Problem: x[4,2048,1024] @ W_attn[1024,3072] -> qkv; 16-head causal attention;
ctx @ W_proj[1024,1024]. f32 I/O.

Sharding (8 cores): data parallel over batch (4) x tensor parallel over heads
(2 groups of 8). Each core computes, for its (b, g):
  - qkv^T for its 8 heads from the full x_b (on-chip transpose of x)
  - causal attention in transposed-score layout: S^T[k,q] blocks, exp on ACT
    (no max subtraction: scores ~ N(0,1), exp never overflows fp32),
    causal mask via affine_select, softmax denominator via a ones-column
    appended to V (ctx^T row 64 accumulates sum_k P), normalize ctx^T columns
  - partial out = ctx_g @ W_proj[g*512:(g+1)*512, :]  (bf16 [2048,1024])
Host gather: out_b = partial(b,0) + partial(b,1) + b_proj.

All tensor args shipped as bf16 (matmul compute dtype is bf16 anyway); the
per-call host<->device transfer through the tunnel dominates wall time, so
bytes-per-call is the primary cost.
"""

import sys

if "/opt/trn_rl_repo" not in sys.path:
    sys.path.insert(0, "/opt/trn_rl_repo")

from contextlib import ExitStack

import numpy as np
import ml_dtypes

import concourse.mybir as mybir
import concourse.tile as tile
from concourse import bacc
from concourse.masks import make_identity

# geometry (hardcoded for this problem)
B, S, D = 4, 2048, 1024
H, HD = 16, 64
NCORES = 8
NH = 8            # heads per core
GC = D // 2       # per-core qkv col split = 512
ST = S // 128     # seq tiles of 128
DC = D // 128     # contraction chunks of 128
NQ = S // 512     # 512-wide q chunks
SCALE = 1.0 / np.sqrt(HD)

MDT = mybir.dt.bfloat16  # matmul compute dtype
F32 = mybir.dt.float32
BF16 = ml_dtypes.bfloat16


def build_bass(phase="D"):
    nc = bacc.Bacc()
    dt = mybir.dt

    x = nc.declare_dram_parameter("x", [S, D], MDT, isOutput=False)
    wq = nc.declare_dram_parameter("wq", [D, GC], MDT, isOutput=False)
    wk = nc.declare_dram_parameter("wk", [D, GC], MDT, isOutput=False)
    wv = nc.declare_dram_parameter("wv", [D, GC], MDT, isOutput=False)
    bqkv = nc.declare_dram_parameter("bqkv", [3 * GC], F32, isOutput=False)  # [bq|bk|bv]
    wp = nc.declare_dram_parameter("wp", [GC, D], MDT, isOutput=False)
    o = nc.declare_dram_parameter("o", [S, D], MDT, isOutput=True)
    wmats = {"q": wq, "k": wk, "v": wv}

    with ExitStack() as top:
        tc = top.enter_context(tile.TileContext(nc))

        const = top.enter_context(tc.tile_pool(name="const", bufs=1))
        ident = const.tile([128, 128], MDT)
        make_identity(nc, ident[:])

        # bias tiles: one [128,1] per (mat, col-block)
        bias_t = {}
        for mi, mat in enumerate(("q", "k", "v")):
            for cb in range(GC // 128):
                t = const.tile([128, 1], F32, tag=f"bias_{mat}{cb}")
                nc.sync.dma_start(t[:], bqkv[mi * GC + cb * 128 : mi * GC + (cb + 1) * 128].rearrange("(p one) -> p one", one=1))
                bias_t[(mat, cb)] = t

        # persistent SBUF data
        data = top.enter_context(tc.tile_pool(name="data", bufs=1))
        qT = [data.tile([64, S], MDT, tag=f"qT{h}", name=f"qT{h}") for h in range(NH)]
        kT = [data.tile([64, S], MDT, tag=f"kT{h}", name=f"kT{h}") for h in range(NH)]
        vst = [data.tile([128, NH * (HD + 1)], MDT, tag=f"vst{s}", name=f"vst{s}") for s in range(ST)]
        ctxT = [data.tile([128, S], MDT, tag=f"ctxT{t}", name=f"ctxT{t}") for t in range(GC // 128)]

        # ---------------- phase A: x -> xT (bf16) ----------------
        xtv_cm = tc.tile_pool(name="xtv", bufs=1)
        xtv = xtv_cm.__enter__()
        xT = [xtv.tile([128, S], MDT, tag=f"xT{c}", name=f"xT{c}") for c in range(DC)]
        vT = [xtv.tile([128, S], MDT, tag=f"vT{cb}", name=f"vT{cb}") for cb in range(GC // 128)]
        with tc.tile_pool(name="tps", bufs=4, space="PSUM") as tps:
            with tc.tile_pool(name="xin", bufs=3) as xin:
                for s in range(ST):
                    x_bf = xin.tile([128, D], MDT, tag="x_bf")
                    nc.sync.dma_start(x_bf[:], x[s * 128 : (s + 1) * 128, :])
                    for c in range(DC):
                        psA = tps.tile([128, 128], MDT, tag="psA")
                        nc.tensor.transpose(psA[:], x_bf[:, c * 128 : (c + 1) * 128], ident[:])
                        if c % 2 == 0:
                            nc.scalar.copy(xT[c][:, s * 128 : (s + 1) * 128], psA[:])
                        else:
                            nc.vector.tensor_copy(xT[c][:, s * 128 : (s + 1) * 128], psA[:])

            # ---------------- phase B: QKV^T ----------------
            with tc.tile_pool(name="wpool", bufs=DC + 2) as wpool, \
                 tc.tile_pool(name="qkvps", bufs=3, space="PSUM") as qkvps_pool:
                for mat in ("q", "k", "v"):
                    for cb in range(GC // 128):
                        wbf = []
                        for d in range(DC):
                            wb = wpool.tile([128, 128], MDT, tag="wbf")
                            nc.sync.dma_start(wb[:], wmats[mat][d * 128 : (d + 1) * 128, cb * 128 : (cb + 1) * 128])
                            wbf.append(wb)
                        for n in range(S // 512):
                            ps = qkvps_pool.tile([128, 512], F32, tag="qkvps")
                            for d in range(DC):
                                nc.tensor.matmul(
                                    ps[:], wbf[d][:], xT[d][:, n * 512 : (n + 1) * 512],
                                    start=(d == 0), stop=(d == DC - 1),
                                )
                            span = (slice(None), slice(n * 512, (n + 1) * 512))
                            bt = bias_t[(mat, cb)]
                            if mat == "v":
                                nc.scalar.activation(
                                    vT[cb][span], ps[:],
                                    mybir.ActivationFunctionType.Identity,
                                    bias=bt[:], scale=1.0,
                                )
                            else:
                                dstl = qT if mat == "q" else kT
                                for half in range(2):
                                    h = 2 * cb + half
                                    nc.scalar.activation(
                                        dstl[h][span],
                                        ps[64 * half : 64 * half + 64, :],
                                        mybir.ActivationFunctionType.Identity,
                                        bias=bt[64 * half : 64 * half + 64, :], scale=1.0,
                                    )

            # ---------------- phase B2: vT -> V natural with ones cols ----------------
            for s in range(ST):
                nc.gpsimd.memset(vst[s].rearrange("p (h u) -> p h u", u=HD + 1)[:, :, HD : HD + 1], 1.0)
            for cb in range(GC // 128):
                for s in range(ST):
                    psB = tps.tile([128, 128], MDT, tag="psA")
                    nc.tensor.transpose(psB[:], vT[cb][:, s * 128 : (s + 1) * 128], ident[:])
                    for half in range(2):
                        h = 2 * cb + half
                        eng = nc.vector.tensor_copy if half == 0 else nc.scalar.copy
                        eng(
                            vst[s][:, h * (HD + 1) : h * (HD + 1) + HD],
                            psB[:, 64 * half : 64 * half + 64],
                        )
        xtv_cm.__exit__(None, None, None)

        if phase == "B":
            with tc.tile_pool(name="dbg", bufs=2) as dbg:
                for s in range(ST):
                    d = dbg.tile([128, NH * (HD + 1)], MDT, tag="dbgt")
                    nc.scalar.copy(d[:], vst[s][:])
                    nc.sync.dma_start(o[s * 128 : (s + 1) * 128, 0 : NH * (HD + 1)], d[:])

        # ---------------- phase C: attention per head ----------------
        with tc.tile_pool(name="spool", bufs=3, space="PSUM") as spool, \
             tc.tile_pool(name="cpool", bufs=5, space="PSUM") as cpool, \
             tc.tile_pool(name="ppool", bufs=6) as ppool, \
             tc.tile_pool(name="npool", bufs=4) as npool:
            for h in range(NH if phase in ("C", "D") else 0):
                ctxps = [cpool.tile([65, 512], F32, tag="ctxps", name="ctxps") for _ in range(NQ)]
                for j in range(ST):
                    c0 = j // 4
                    for c in range(c0, NQ):
                        qoff = 128 * j if c == c0 else 512 * c
                        qlen = 512 * (c + 1) - qoff
                        sps = spool.tile([128, 512], F32, tag="sps")
                        nc.tensor.matmul(
                            sps[:, :qlen],
                            kT[h][:, j * 128 : (j + 1) * 128],
                            qT[h][:, qoff : qoff + qlen],
                            start=True, stop=True,
                        )
                        pT = ppool.tile([128, 512], MDT, tag="pT")
                        nc.scalar.activation(
                            pT[:, :qlen], sps[:, :qlen],
                            mybir.ActivationFunctionType.Exp,
                            bias=0.0, scale=float(SCALE),
                        )
                        if c == c0:
                            # diagonal block: keep q >= k  (q = qoff+f = 128j+f, k = 128j+p)
                            nc.gpsimd.affine_select(
                                out=pT[:, 0:128], in_=pT[:, 0:128],
                                compare_op=mybir.AluOpType.is_ge,
                                fill=0.0, base=0,
                                pattern=[[1, 128]], channel_multiplier=-1,
                            )
                        nc.tensor.matmul(
                            ctxps[c][:, qoff - 512 * c : qoff - 512 * c + qlen],
                            vst[j][:, h * (HD + 1) : (h + 1) * (HD + 1)],
                            pT[:, :qlen],
                            start=(j == 0), stop=(j == 4 * c + 3),
                        )
                        if j == 4 * c + 3:
                            # chunk complete -> normalize columns by 1/denominator
                            rec1 = npool.tile([1, 512], F32, tag="rec1")
                            nc.vector.reciprocal(rec1[:], ctxps[c][64:65, :])
                            recb = npool.tile([64, 512], F32, tag="recb")
                            nc.gpsimd.partition_broadcast(recb[:], rec1[:])
                            nc.vector.tensor_mul(
                                ctxT[h // 2][64 * (h % 2) : 64 * (h % 2) + 64, c * 512 : (c + 1) * 512],
                                ctxps[c][0:64, :],
                                recb[:],
                            )

        if phase == "C":
            with tc.tile_pool(name="dbg2", bufs=2) as dbg2:
                for t in range(GC // 128):
                    for s in range(ST):
                        d = dbg2.tile([128, 128], MDT, tag="dbg2t")
                        nc.scalar.copy(d[:], ctxT[t][:, s * 128 : (s + 1) * 128])
                        nc.sync.dma_start(o[s * 128 : (s + 1) * 128, t * 128 : (t + 1) * 128], d[:])

        # ---------------- phase D: out = ctx @ Wp ----------------
        with tc.tile_pool(name="wpstp", bufs=2) as wpstp, \
             tc.tile_pool(name="wpbp", bufs=1) as wpbp, \
             tc.tile_pool(name="opool", bufs=2) as opool, \
             tc.tile_pool(name="pps", bufs=3, space="PSUM") as pps_pool:
            if phase != "D":
                wz = wpstp.tile([128, 8], MDT, tag="wz")
                nc.sync.dma_start(wz[:], wp[0:128, 0:8])
                nc.sync.dma_start(o[0:128, 0:8], wz[:])
            wpb = []
            for t in range(GC // 128 if phase == "D" else 0):
                wb = wpbp.tile([128, D], MDT, tag=f"wpb{t}", name=f"wpb{t}")
                nc.sync.dma_start(wb[:], wp[t * 128 : (t + 1) * 128, :])
                wpb.append(wb)
            for m in range(ST if phase == "D" else 0):
                out_sb = opool.tile([128, D], MDT, tag="out_sb")
                for nb in range(2):
                    ps = pps_pool.tile([128, 512], F32, tag="pps")
                    for t in range(GC // 128):
                        nc.tensor.matmul(
                            ps[:],
                            ctxT[t][:, m * 128 : (m + 1) * 128],
                            wpb[t][:, nb * 512 : (nb + 1) * 512],
                            start=(t == 0), stop=(t == GC // 128 - 1),
                        )
                    nc.scalar.copy(out_sb[:, nb * 512 : (nb + 1) * 512], ps[:])
                nc.sync.dma_start(o[m * 128 : (m + 1) * 128, :], out_sb[:])

    nc.finalize()
    return nc


_NC_CACHE = None


def get_nc():
    global _NC_CACHE
    if _NC_CACHE is None:
        _NC_CACHE = build_bass()
    return _NC_CACHE


def make_in_maps(x, W_attn, b_attn, W_proj):
    x = np.asarray(x, dtype=np.float32).astype(BF16)
    W_attn = np.asarray(W_attn, dtype=np.float32).astype(BF16)
    b_attn = np.asarray(b_attn, dtype=np.float32)
    W_proj = np.asarray(W_proj, dtype=np.float32).astype(BF16)
    in_maps = []
    for core in range(NCORES):
        b, g = divmod(core, 2)
        cs = slice(g * GC, (g + 1) * GC)
        in_maps.append({
            "x": np.ascontiguousarray(x[b]),
            "wq": np.ascontiguousarray(W_attn[:, 0 * D :][:, cs]),
            "wk": np.ascontiguousarray(W_attn[:, 1 * D :][:, cs]),
            "wv": np.ascontiguousarray(W_attn[:, 2 * D :][:, cs]),
            "bqkv": np.ascontiguousarray(np.concatenate(
                [b_attn[0 * D :][cs], b_attn[1 * D :][cs], b_attn[2 * D :][cs]]
            )),
            "wp": np.ascontiguousarray(W_proj[g * GC : (g + 1) * GC, :]),
        })
    return in_maps


def gather_output(results, b_proj):
    out = np.empty((B, S, D), dtype=np.float32)
    for b in range(B):
        out[b] = results[2 * b]["o"].astype(np.float32) + results[2 * b + 1]["o"].astype(np.float32)
    out += np.asarray(b_proj, dtype=np.float32)[None, None, :]
    return out


def kernel(x, W_attn, b_attn, W_proj, b_proj):
    from concourse.bass_utils import run_bass_kernel_spmd

    nc = get_nc()
    in_maps = make_in_maps(x, W_attn, b_attn, W_proj)
    res = run_bass_kernel_spmd(nc, in_maps, list(range(NCORES)))
    return gather_output(res.results, b_proj)


# revision 14
# speedup vs baseline: 18.5016x; 9.6284x over previous
"""Causal self-attention Trainium2 kernel (8 NeuronCores).

Problem: x[4,2048,1024] @ W_attn[1024,3072] -> qkv; 16-head causal attention;
ctx @ W_proj[1024,1024]. f32 I/O.

Sharding (8 cores): data parallel over batch (4) x tensor parallel over heads
(2 groups of 8). Per-call host<->device transfer dominates wall time, so the
host ships each byte once (bf16) and the cores deduplicate over NeuronLink:
  - core (b,g) ships rows g*1024:(g+1)*1024 of x_b; pair AllGather -> full x_b
  - core (b,g) ships ONE weight matrix of group g's set (b=0 -> wq_g,
    1 -> wk_g, 2 -> wv_g, 3 -> wp_g; each is exactly 524288 elements);
    4-way AllGather across same-g cores -> full weight set
  - each core computes the full partial out_b = ctx_g @ W_proj[g*512:,:];
    two pair ReduceScatters (one per 1024-row half, overlapping phase D)
    leave each core rows g*512:(g+1)*512 of each half
Host gather: out[b] = interleave of the four 512-row chunks + b_proj.
All inputs ship as ONE merged bf16 buffer per core ([x_half | wshard | bqkv]);
per-call transfer (bytes and buffer count) dominates wall time on this rig.

Compute per core: qkv^T for 8 heads (on-chip transpose of x), causal
attention in transposed-score layout (exp on ACT, causal mask via
affine_select, softmax denominator via ones-column appended to V), then
ctx_g @ wp_g. Matmul compute dtype bf16 (fp32 PSUM accumulate).
"""

import sys

if "/opt/trn_rl_repo" not in sys.path:
    sys.path.insert(0, "/opt/trn_rl_repo")

from contextlib import ExitStack

import numpy as np
import ml_dtypes

import concourse.mybir as mybir
import concourse.tile as tile
from concourse import bacc
from concourse.masks import make_identity

# geometry (hardcoded for this problem)
B, S, D = 4, 2048, 1024
H, HD = 16, 64
NCORES = 8
NH = 8            # heads per core
GC = D // 2       # per-core qkv col split = 512
ST = S // 128     # seq tiles of 128
DC = D // 128     # contraction chunks of 128
NQ = S // 512     # 512-wide q chunks
SH = S // 2       # seq half = 1024
WQE = D * GC      # elements per weight matrix shard = 524288
SCALE = 1.0 / np.sqrt(HD)

MDT = mybir.dt.bfloat16  # matmul compute dtype
F32 = mybir.dt.float32
BF16 = ml_dtypes.bfloat16

PAIRS = [[0, 1], [2, 3], [4, 5], [6, 7]]
G4 = [[0, 2, 4, 6], [1, 3, 5, 7]]


def build_bass(phase="D"):
    nc = bacc.Bacc(num_devices=NCORES)
    dt = mybir.dt

    # single merged input buffer: [x_half | wshard | bqkv] — per-call cost has
    # a large fixed per-buffer component, so ship one buffer, not three
    xin = nc.declare_dram_parameter("xin", [SH * D + WQE + 3 * GC], MDT, isOutput=False)
    o = nc.declare_dram_parameter("o", [SH, D], MDT, isOutput=True)
    x = xin[0 : SH * D].rearrange("(r c) -> r c", c=D)
    wshard = xin[SH * D : SH * D + WQE]
    bqkv = xin[SH * D + WQE : SH * D + WQE + 3 * GC]  # [bq|bk|bv] bf16

    with ExitStack() as top:
        tc = top.enter_context(tile.TileContext(nc))

        # ---------------- DRAM bounce buffers + collectives ----------------
        dram = top.enter_context(tc.tile_pool(name="dram", bufs=1, space="DRAM"))
        x_inb = dram.tile([SH, D], MDT)
        xg = dram.tile([S, D], MDT)
        w_inb = dram.tile([WQE], MDT)
        wg = dram.tile([4 * WQE], MDT)
        opart = dram.tile([S, D], MDT)
        ored = dram.tile([SH, D], MDT)

        nc.gpsimd.dma_start(x_inb[:], x[:])
        nc.gpsimd.dma_start(w_inb[:], wshard[:])
        nc.gpsimd.collective_compute(
            "AllGather", mybir.AluOpType.bypass,
            replica_groups=PAIRS,
            ins=[x_inb.opt()], outs=[xg.opt()],
        )
        nc.gpsimd.collective_compute(
            "AllGather", mybir.AluOpType.bypass,
            replica_groups=G4,
            ins=[w_inb.opt()], outs=[wg.opt()],
        )
        # full weight views out of the gathered pack
        wq = wg[0 * WQE : 1 * WQE].rearrange("(r c) -> r c", c=GC)   # [1024, 512]
        wk = wg[1 * WQE : 2 * WQE].rearrange("(r c) -> r c", c=GC)
        wv = wg[2 * WQE : 3 * WQE].rearrange("(r c) -> r c", c=GC)
        wp = wg[3 * WQE : 4 * WQE].rearrange("(r c) -> r c", c=D)    # [512, 1024]
        wmats = {"q": wq, "k": wk, "v": wv}

        const = top.enter_context(tc.tile_pool(name="const", bufs=1))
        ident = const.tile([128, 128], MDT)
        make_identity(nc, ident[:])

        # bias tiles: one [128,1] per (mat, col-block); shipped bf16, used f32
        bias_t = {}
        for mi, mat in enumerate(("q", "k", "v")):
            for cb in range(GC // 128):
                tb = const.tile([128, 1], MDT, tag=f"biasb_{mat}{cb}")
                nc.sync.dma_start(tb[:], bqkv[mi * GC + cb * 128 : mi * GC + (cb + 1) * 128].rearrange("(p one) -> p one", one=1))
                t = const.tile([128, 1], F32, tag=f"bias_{mat}{cb}")
                nc.vector.tensor_copy(t[:], tb[:])
                bias_t[(mat, cb)] = t

        # persistent SBUF data; q/k head pairs share a [128, S] tile so phase B
        # can write both with one full-width activation
        data = top.enter_context(tc.tile_pool(name="data", bufs=1))
        qT2 = [data.tile([128, S], MDT, tag=f"qT2{t}", name=f"qT2{t}") for t in range(NH // 2)]
        kT2 = [data.tile([128, S], MDT, tag=f"kT2{t}", name=f"kT2{t}") for t in range(NH // 2)]
        qT = [qT2[h // 2][64 * (h % 2) : 64 * (h % 2) + 64, :] for h in range(NH)]
        kT = [kT2[h // 2][64 * (h % 2) : 64 * (h % 2) + 64, :] for h in range(NH)]
        vst = [data.tile([128, NH * (HD + 1)], MDT, tag=f"vst{s}", name=f"vst{s}") for s in range(ST)]
        ctxT = [data.tile([128, S], MDT, tag=f"ctxT{t}", name=f"ctxT{t}") for t in range(GC // 128)]

        # ---------------- phase A: x -> xT (bf16) ----------------
        xtv_cm = tc.tile_pool(name="xtv", bufs=1)
        xtv = xtv_cm.__enter__()
        xT = [xtv.tile([128, S], MDT, tag=f"xT{c}", name=f"xT{c}") for c in range(DC)]
        vT = [xtv.tile([128, S], MDT, tag=f"vT{cb}", name=f"vT{cb}") for cb in range(GC // 128)]
        with tc.tile_pool(name="tps", bufs=4, space="PSUM") as tps:
            with tc.tile_pool(name="xin", bufs=3) as xin:
                for s in range(ST):
                    x_bf = xin.tile([128, D], MDT, tag="x_bf")
                    nc.sync.dma_start(x_bf[:], xg[s * 128 : (s + 1) * 128, :])
                    for c in range(DC):
                        psA = tps.tile([128, 128], MDT, tag="psA")
                        nc.tensor.transpose(psA[:], x_bf[:, c * 128 : (c + 1) * 128], ident[:])
                        eng = (nc.vector.tensor_copy, nc.vector.tensor_copy, nc.scalar.copy)[c % 3]
                        eng(xT[c][:, s * 128 : (s + 1) * 128], psA[:])

            # ---------------- phase B: QKV^T ----------------
            with tc.tile_pool(name="wpool", bufs=DC + 2) as wpool, \
                 tc.tile_pool(name="qkvps", bufs=3, space="PSUM") as qkvps_pool:
                for mat in ("q", "k", "v"):
                    for cb in range(GC // 128):
                        wbf = []
                        for d in range(DC):
                            wb = wpool.tile([128, 128], MDT, tag="wbf")
                            nc.sync.dma_start(wb[:], wmats[mat][d * 128 : (d + 1) * 128, cb * 128 : (cb + 1) * 128])
                            wbf.append(wb)
                        for n in range(S // 512):
                            ps = qkvps_pool.tile([128, 512], F32, tag="qkvps")
                            for d in range(DC):
                                nc.tensor.matmul(
                                    ps[:], wbf[d][:], xT[d][:, n * 512 : (n + 1) * 512],
                                    start=(d == 0), stop=(d == DC - 1),
                                )
                            span = (slice(None), slice(n * 512, (n + 1) * 512))
                            bt = bias_t[(mat, cb)]
                            dst = vT if mat == "v" else (qT2 if mat == "q" else kT2)
                            nc.scalar.activation(
                                dst[cb][span], ps[:],
                                mybir.ActivationFunctionType.Identity,
                                bias=bt[:], scale=1.0,
                            )

            # ---------------- phase B2: vT -> V natural with ones cols ----------------
            for s in range(ST):
                nc.gpsimd.memset(vst[s].rearrange("p (h u) -> p h u", u=HD + 1)[:, :, HD : HD + 1], 1.0)
            for cb in range(GC // 128):
                for s in range(ST):
                    psB = tps.tile([128, 128], MDT, tag="psA")
                    nc.tensor.transpose(psB[:], vT[cb][:, s * 128 : (s + 1) * 128], ident[:])
                    for half in range(2):
                        h = 2 * cb + half
                        eng = nc.vector.tensor_copy if half == 0 else nc.scalar.copy
                        eng(
                            vst[s][:, h * (HD + 1) : h * (HD + 1) + HD],
                            psB[:, 64 * half : 64 * half + 64],
                        )
        xtv_cm.__exit__(None, None, None)

        # ---------------- phase C: attention per head ----------------
        with tc.tile_pool(name="spool", bufs=3, space="PSUM") as spool, \
             tc.tile_pool(name="cpool", bufs=5, space="PSUM") as cpool, \
             tc.tile_pool(name="ppool", bufs=6) as ppool, \
             tc.tile_pool(name="npool", bufs=4) as npool:
            for h in range(NH if phase in ("C", "D") else 0):
                ctxps = [cpool.tile([65, 512], F32, tag="ctxps", name="ctxps") for _ in range(NQ)]
                for j in range(ST):
                    c0 = j // 4
                    for c in range(c0, NQ):
                        qoff = 128 * j if c == c0 else 512 * c
                        qlen = 512 * (c + 1) - qoff
                        sps = spool.tile([128, 512], F32, tag="sps")
                        nc.tensor.matmul(
                            sps[:, :qlen],
                            kT[h][:, j * 128 : (j + 1) * 128],
                            qT[h][:, qoff : qoff + qlen],
                            start=True, stop=True,
                        )
                        pT = ppool.tile([128, 512], MDT, tag="pT")
                        nc.scalar.activation(
                            pT[:, :qlen], sps[:, :qlen],
                            mybir.ActivationFunctionType.Exp,
                            bias=0.0, scale=float(SCALE),
                        )
                        if c == c0:
                            # diagonal block: keep q >= k  (q = qoff+f = 128j+f, k = 128j+p)
                            nc.gpsimd.affine_select(
                                out=pT[:, 0:128], in_=pT[:, 0:128],
                                compare_op=mybir.AluOpType.is_ge,
                                fill=0.0, base=0,
                                pattern=[[1, 128]], channel_multiplier=-1,
                            )
                        nc.tensor.matmul(
                            ctxps[c][:, qoff - 512 * c : qoff - 512 * c + qlen],
                            vst[j][:, h * (HD + 1) : (h + 1) * (HD + 1)],
                            pT[:, :qlen],
                            start=(j == 0), stop=(j == 4 * c + 3),
                        )
                        if j == 4 * c + 3:
                            # chunk complete -> normalize columns by 1/denominator
                            rec1 = npool.tile([1, 512], F32, tag="rec1")
                            nc.vector.reciprocal(rec1[:], ctxps[c][64:65, :])
                            recb = npool.tile([64, 512], F32, tag="recb")
                            nc.gpsimd.partition_broadcast(recb[:], rec1[:])
                            nc.vector.tensor_mul(
                                ctxT[h // 2][64 * (h % 2) : 64 * (h % 2) + 64, c * 512 : (c + 1) * 512],
                                ctxps[c][0:64, :],
                                recb[:],
                            )

        # ---------------- phase D: out = ctx @ Wp -> ReduceScatter ----------------
        with tc.tile_pool(name="wpbp", bufs=1) as wpbp, \
             tc.tile_pool(name="opool", bufs=2) as opool, \
             tc.tile_pool(name="pps", bufs=3, space="PSUM") as pps_pool:
            wpb = []
            for t in range(GC // 128):
                wb = wpbp.tile([128, D], MDT, tag=f"wpb{t}", name=f"wpb{t}")
                nc.sync.dma_start(wb[:], wp[t * 128 : (t + 1) * 128, :])
                wpb.append(wb)
            for m in range(ST):
                out_sb = opool.tile([128, D], MDT, tag="out_sb")
                for nb in range(2):
                    ps = pps_pool.tile([128, 512], F32, tag="pps")
                    for t in range(GC // 128):
                        nc.tensor.matmul(
                            ps[:],
                            ctxT[t][:, m * 128 : (m + 1) * 128],
                            wpb[t][:, nb * 512 : (nb + 1) * 512],
                            start=(t == 0), stop=(t == GC // 128 - 1),
                        )
                    nc.vector.tensor_copy(out_sb[:, nb * 512 : (nb + 1) * 512], ps[:])
                nc.sync.dma_start(opart[m * 128 : (m + 1) * 128, :], out_sb[:])
                if m == ST // 2 - 1 or m == ST - 1:
                    # ReduceScatter each half as soon as phase D finishes it so
                    # the first RS overlaps the second half's matmuls. Core g
                    # ends up with rows [g*512:(g+1)*512] of each half; the
                    # host reassembles (see gather_output).
                    half = m // (ST // 2)
                    nc.gpsimd.collective_compute(
                        "ReduceScatter", mybir.AluOpType.add,
                        replica_groups=PAIRS,
                        ins=[opart[half * SH : (half + 1) * SH, :].opt()],
                        outs=[ored[half * (SH // 2) : half * (SH // 2) + SH // 2, :].opt()],
                    )
        nc.gpsimd.dma_start(o[:], ored[:])

    nc.finalize()
    return nc


_NC_CACHE = None


def get_nc():
    global _NC_CACHE
    if _NC_CACHE is None:
        _NC_CACHE = build_bass()
    return _NC_CACHE


def make_in_maps(x, W_attn, b_attn, W_proj):
    x = np.asarray(x, dtype=np.float32).astype(BF16)
    W_attn = np.asarray(W_attn, dtype=np.float32).astype(BF16)
    b_attn = np.asarray(b_attn, dtype=np.float32).astype(BF16)
    W_proj = np.asarray(W_proj, dtype=np.float32).astype(BF16)
    in_maps = []
    for core in range(NCORES):
        b, g = divmod(core, 2)
        cs = slice(g * GC, (g + 1) * GC)
        if b == 3:
            wsh = W_proj[g * GC : (g + 1) * GC, :]
        else:
            wsh = W_attn[:, b * D :][:, cs]
        in_maps.append({
            "xin": np.concatenate([
                x[b, g * SH : (g + 1) * SH, :].reshape(-1),
                np.ascontiguousarray(wsh).reshape(-1),
                b_attn[0 * D :][cs], b_attn[1 * D :][cs], b_attn[2 * D :][cs],
            ]),
        })
    return in_maps


def gather_output(results, b_proj):
    # o[core 2b+g] = [rows g*512:(g+1)*512 of out_b[:1024] ; same rows of out_b[1024:]]
    out = np.empty((B, S, D), dtype=np.float32)
    Q = SH // 2
    for b in range(B):
        o0 = results[2 * b]["o"].astype(np.float32)
        o1 = results[2 * b + 1]["o"].astype(np.float32)
        out[b, 0 * Q : 1 * Q] = o0[:Q]
        out[b, 1 * Q : 2 * Q] = o1[:Q]
        out[b, 2 * Q : 3 * Q] = o0[Q:]
        out[b, 3 * Q : 4 * Q] = o1[Q:]
    out += np.asarray(b_proj, dtype=np.float32)[None, None, :]
    return out


def kernel(x, W_attn, b_attn, W_proj, b_proj):
    from concourse.bass_utils import run_bass_kernel_spmd

    nc = get_nc()
    in_maps = make_in_maps(x, W_attn, b_attn, W_proj)
    res = run_bass_kernel_spmd(nc, in_maps, list(range(NCORES)))
    return gather_output(res.results, b_proj)
